# revision 14
# baseline (speedup 1.0000x reference)
"""CABlock (cross-attention block) Trainium2 Bass kernel.

Problem: b=8, c=64, h=w=48 (n=2304), CR=8.
  qk_i = Wqk_i @ x_i + bqk_i  (q = first 8 rows, k = last 8)
  attn_i = softmax_j(q_i^T k_i)            [n, n]
  o1 = (Wv1@x1 + bv1) @ attn2 * gamma + x1
  o2 = (Wv2@x2 + bv2) @ attn1 * beta  + x2

Sharding: data-parallel over batch, 1 batch element per NeuronCore (8 cores).

Per-core dataflow (channel-on-partition):
  - x packed [128, 2304] fp32r: x1 @ partitions 0:64, x2 @ 64:128.
  - q/k [128, 2*2304] fp32r at partitions 0:8 (attn1 cols 0:N, attn2 cols
    N:2N).  Logit matmuls (K=8) run in fp32r (full PE rate for N>=256, vs
    4x slower plain fp32); fp32r requires dst psum partition base 0.
  - A-row (128 queries x 2304 keys) computed in 512-col PSUM chunks through
    3 rotating single-bank tiles; ScalarE exp's each chunk PSUM->SBUF(bf16)
    with accum_out giving softmax row-sums for free.  No max subtraction
    (logit range is far inside fp32 exp range).
  - 1/s (and gamma/beta) folded into the tiny [128, 64] V^T tiles instead of
    the n x n matrix.  E and V^T in bf16 -> o-matmuls run at full rate and
    may use column tiling (o2 at psum partitions 64:128); the attention term
    is scaled by gamma/beta = 0.1, so bf16 rounding there is ~5e-4 relative
    to the residual-dominated output.
  - o1/o2 column-packed into one PSUM accumulator [128, 2304] (5 banks),
    accumulated over all 18 i-tiles via start/stop; final residual add on DVE.
"""

import numpy as np

C = 64
CR = 8
H = W = 48
N = H * W            # 2304
B = 8
P = 128
IT = N // P          # 18 i-tiles
CHUNKS = [(0, 512), (512, 512), (1024, 512), (1536, 512), (2048, 256)]
NCHUNK = len(CHUNKS)

_CACHE = {}


def _build():
    import concourse.bacc as bacc
    import concourse.tile as tile
    from concourse import mybir

    F32 = mybir.dt.float32
    F32R = mybir.dt.float32r
    BF16 = mybir.dt.bfloat16
    AF = mybir.ActivationFunctionType
    ALU = mybir.AluOpType
    AX = mybir.AxisListType

    nc = bacc.Bacc("TRN2", target_bir_lowering=False, debug=False, num_devices=8)

    x1_d = nc.dram_tensor("x1", [C, N], F32R, kind="ExternalInput")
    x2_d = nc.dram_tensor("x2", [C, N], F32R, kind="ExternalInput")
    # consts columns: 0:8 wqT, 8:16 wkT, 16:80 wvT, 80 q1bias, 81 k1bias,
    # 82 q2bias, 83 k2bias (rows 0:8), 84:148 bv1 bcast, 148:212 bv2 bcast,
    # 212 gamma, 213 beta
    cst_d = nc.dram_tensor("consts", [P, 214], F32R, kind="ExternalInput")
    out_d = nc.dram_tensor("out", [P, N], F32, kind="ExternalOutput")

    with tile.TileContext(nc) as tc:
        with (
            tc.tile_pool(name="big", bufs=1) as big,
            tc.tile_pool(name="epool", bufs=3) as epool,
            tc.tile_pool(name="small", bufs=4) as small,
            tc.tile_pool(name="psA", bufs=3, space="PSUM") as psA,
            tc.tile_pool(name="psO", bufs=1, space="PSUM") as psO,
        ):
            # ---- early ACT table warm (loads exp tables during DMA wait)
            warm = big.tile([P, 1], F32, name="warm", tag="warm")
            warm2 = big.tile([P, 1], F32, name="warm2", tag="warm2")
            nc.vector.memset(warm, 0.0)
            nc.scalar.activation(out=warm2, in_=warm, func=AF.Exp)

            # ---- constant + input DMAs
            cst = big.tile([P, 214], F32R, name="cst", tag="cst")
            nc.sync.dma_start(out=cst, in_=cst_d.ap())
            x_sb = big.tile([P, N], F32R, name="x_sb", tag="x_sb")
            nc.sync.dma_start(out=x_sb[0:C, :], in_=x1_d.ap())
            nc.sync.dma_start(out=x_sb[C:P, :], in_=x2_d.ap())

            wq = cst[:, 0:8]
            wk = cst[:, 8:16]
            wv = cst[:, 16:80]
            qkbias = [[cst[:, 80:81].bitcast(F32), cst[:, 81:82].bitcast(F32)],
                      [cst[:, 82:83].bitcast(F32), cst[:, 83:84].bitcast(F32)]]
            bvb = [cst[:, 84:148].bitcast(F32), cst[:, 148:212].bitcast(F32)]
            gamma = cst[:, 212:213].bitcast(F32)
            beta = cst[:, 213:214].bitcast(F32)

            # q/k for both attns at partitions 0:8; attn an at cols an*N
            q_sb = big.tile([P, 2 * N], F32R, name="q_sb", tag="q_sb")
            k_sb = big.tile([P, 2 * N], F32R, name="k_sb", tag="k_sb")
            vt1b = big.tile([P, IT * C], F32, name="vt1b", tag="vt1b")
            vt2b = big.tile([P, IT * C], F32, name="vt2b", tag="vt2b")
            out_sb = big.tile([P, N], F32, name="out_sb", tag="out_sb")

            # ---- projection phase per column chunk: q/k then V^T tiles
            for ci, (off, w) in enumerate(CHUNKS):
                for an in (0, 1):  # attn index
                    xs = x_sb[0:C, off:off + w] if an == 0 else x_sb[C:P, off:off + w]
                    wqs = wq[0:C, :] if an == 0 else wq[C:P, :]
                    wks = wk[0:C, :] if an == 0 else wk[C:P, :]
                    pq = ps_tile(f"pq{ci}_{an}")
                    nc.tensor.matmul(pq[0:8, :w], wqs, xs)
                    nc.vector.tensor_scalar(
                        out=q_sb[0:8, an * N + off:an * N + off + w],
                        in0=pq[0:8, :w],
                        scalar1=qkbias[an][0][0:8, :], scalar2=None, op0=ALU.add)
                    pk = ps_tile(f"pk{ci}_{an}")
                    nc.tensor.matmul(pk[0:8, :w], wks, xs)
                    nc.vector.tensor_scalar(
                        out=k_sb[0:8, an * N + off:an * N + off + w],
                        in0=pk[0:8, :w],
                        scalar1=qkbias[an][1][0:8, :], scalar2=None, op0=ALU.add)
                # V^T tiles living in this column chunk
                for t in range(4 * ci, min(4 * ci + 4, IT)):
                    sub = (t % 4) * P + off - (t // 4) * 512
                    sub = (t * P) - off
                    pv1 = ps_tile(f"pv1_{t}")
                    nc.tensor.matmul(
                        pv1[:, 0:C], x_sb[0:C, t * P:(t + 1) * P], wv[0:C, :])
                    nc.vector.tensor_tensor(
                        out=vt1b[:, t * C:(t + 1) * C], in0=pv1[:, 0:C], in1=bvb[0],
                        op=ALU.add)
                    pv2 = ps_tile(f"pv2_{t}")
                    nc.tensor.matmul(
                        pv2[:, 0:C], x_sb[C:P, t * P:(t + 1) * P], wv[C:P, :])
                    nc.vector.tensor_tensor(
                        out=vt2b[:, t * C:(t + 1) * C], in0=pv2[:, 0:C], in1=bvb[1],
                        op=ALU.add)

            # ---- main loop over i-tiles
            psum_o = psO.tile([P, N], F32, name="psum_o", tag="pso")

            def emit_omms(t, e1t, e2t, vts):
                st, sp = (t == 0), (t == IT - 1)
                for (off, w) in CHUNKS:
                    # o1 (partitions 0:64) <- vt1s^T @ E2 ; o2 <- vt2s^T @ E1
                    nc.tensor.matmul(
                        psum_o[0:C, off:off + w], vts[:, 0:C],
                        e2t[:, off:off + w], start=st, stop=sp)
                    nc.tensor.matmul(
                        psum_o[C:P, off:off + w], vts[:, C:P],
                        e1t[:, off:off + w], start=st, stop=sp)

            # o-mm emission counts after each of the 10 (an, chunk) A-mm
            # positions: keep PE just ahead of ACT, never a block of o-mms.
            O_COUNTS = [0, 0, 1, 1, 1, 1, 1, 1, 2, 2]

            def omm_list(t, e1t, e2t, vts):
                st, sp = (t == 0), (t == IT - 1)
                mms = []
                for (off, w) in CHUNKS:
                    mms.append((psum_o[0:C, off:off + w], vts[:, 0:C],
                                e2t[:, off:off + w], st, sp))
                    mms.append((psum_o[C:P, off:off + w], vts[:, C:P],
                                e1t[:, off:off + w], st, sp))
                return mms

            prev = None
            for t in range(IT):
                pending = omm_list(*prev) if prev is not None else []
                pi = 0
                ets = []
                sps = []
                for an in (0, 1):
                    qs = slice(an * N + t * P, an * N + (t + 1) * P)
                    et = epool.tile([P, N], BF16, name=f"e{an}_{t}", tag=f"e{an}")
                    sp = small.tile([P, 8], F32, name=f"sp{an}_{t}", tag=f"sp{an}")
                    for ci, (off, w) in enumerate(CHUNKS):
                        pa = ps_tile(f"pa{an}_{t}_{ci}")
                        nc.tensor.matmul(
                            pa[:, :w], q_sb[0:8, qs],
                            k_sb[0:8, an * N + off:an * N + off + w])
                        nc.scalar.activation(
                            out=et[:, off:off + w], in_=pa[:, :w], func=AF.Exp,
                            accum_out=sp[:, ci:ci + 1])
                        for _ in range(O_COUNTS[an * NCHUNK + ci]):
                            if pending:
                                o, l, rr_, st_, sp_ = pending.pop(0)
                                nc.tensor.matmul(o, l, rr_, start=st_, stop=sp_)
                    ets.append(et)
                    sps.append(sp)
                    # row stats as soon as this row's exps are emitted
                    s = small.tile([P, 1], F32, name=f"s{an}_{t}", tag=f"s{an}")
                    nc.vector.tensor_reduce(
                        s, sp[:, 0:NCHUNK], axis=AX.X, op=ALU.add)
                    rr = small.tile([P, 1], F32, name=f"r{an}_{t}", tag=f"r{an}")
                    nc.vector.reciprocal(rr, s)
                    sps.append(rr)
                for o, l, rr_, st_, sp_ in pending:
                    nc.tensor.matmul(o, l, rr_, start=st_, stop=sp_)
                r1_, r2_ = sps[1], sps[3]
                # vts cols 0:64 = vt1b*(1/s2)*gamma ; 64:128 = vt2b*(1/s1)*beta
                vts = small.tile([P, P], BF16, name=f"vts_{t}", tag="vts")
                nc.vector.tensor_scalar(
                    out=vts[:, 0:C], in0=vt1b[:, t * C:(t + 1) * C],
                    scalar1=r2_, scalar2=gamma, op0=ALU.mult, op1=ALU.mult)
                nc.vector.tensor_scalar(
                    out=vts[:, C:P], in0=vt2b[:, t * C:(t + 1) * C],
                    scalar1=r1_, scalar2=beta, op0=ALU.mult, op1=ALU.mult)
                prev = (t, ets[0], ets[1], vts)
            for o, l, rr_, st_, sp_ in omm_list(*prev):
                nc.tensor.matmul(o, l, rr_, start=st_, stop=sp_)

            # ---- final: out = psum_o + x  (residual), chunked store
            for ci, (off, w) in enumerate(CHUNKS):
                nc.vector.tensor_tensor(
                    out=out_sb[:, off:off + w], in0=psum_o[:, off:off + w],
                    in1=x_sb[:, off:off + w].bitcast(F32), op=ALU.add)
                nc.sync.dma_start(
                    out=out_d.ap()[:, off:off + w], in_=out_sb[:, off:off + w])

    nc.compile()
    return nc


def _get_nc():
    if "nc" not in _CACHE:
        _CACHE["nc"] = _build()
    return _CACHE["nc"]


def _make_in_maps(x1, x2, Wqk1, bqk1, Wqk2, bqk2, Wv1, bv1, Wv2, bv2, gamma, beta):
    f = np.float32
    consts = np.zeros((P, 214), dtype=f)
    consts[0:C, 0:8] = np.asarray(Wqk1, f)[0:CR, :].T
    consts[C:P, 0:8] = np.asarray(Wqk2, f)[0:CR, :].T
    consts[0:C, 8:16] = np.asarray(Wqk1, f)[CR:2 * CR, :].T
    consts[C:P, 8:16] = np.asarray(Wqk2, f)[CR:2 * CR, :].T
    consts[0:C, 16:80] = np.asarray(Wv1, f).T
    consts[C:P, 16:80] = np.asarray(Wv2, f).T
    consts[0:CR, 80] = np.asarray(bqk1, f)[0:CR]
    consts[0:CR, 81] = np.asarray(bqk1, f)[CR:2 * CR]
    consts[0:CR, 82] = np.asarray(bqk2, f)[0:CR]
    consts[0:CR, 83] = np.asarray(bqk2, f)[CR:2 * CR]
    consts[:, 84:148] = np.asarray(bv1, f)[None, :]
    consts[:, 148:212] = np.asarray(bv2, f)[None, :]
    consts[:, 212] = np.float32(np.asarray(gamma, f).reshape(-1)[0])
    consts[:, 213] = np.float32(np.asarray(beta, f).reshape(-1)[0])

    x1 = np.ascontiguousarray(np.asarray(x1, f).reshape(B, C, N))
    x2 = np.ascontiguousarray(np.asarray(x2, f).reshape(B, C, N))
    return [
        {"x1": np.ascontiguousarray(x1[i]), "x2": np.ascontiguousarray(x2[i]),
         "consts": consts}
        for i in range(B)
    ]


def _run(in_maps, **kwargs):
    from concourse.bass_utils import run_bass_kernel_spmd
    nc = _get_nc()
    return run_bass_kernel_spmd(nc, in_maps, core_ids=list(range(B)), **kwargs)


def kernel(x1, x2, Wqk1, bqk1, Wqk2, bqk2, Wv1, bv1, Wv2, bv2, gamma, beta):
    in_maps = _make_in_maps(x1, x2, Wqk1, bqk1, Wqk2, bqk2, Wv1, bv1, Wv2, bv2,
                            gamma, beta)
    res = _run(in_maps)
    o1 = np.empty((B, C, H, W), dtype=np.float32)
    o2 = np.empty((B, C, H, W), dtype=np.float32)
    for i in range(B):
        full = res.results[i]["out"]
        o1[i] = full[0:C, :].reshape(C, H, W)
        o2[i] = full[C:P, :].reshape(C, H, W)
    return o1, o2


# revision 15
# speedup vs baseline: 1.0007x; 1.0007x over previous
"""CABlock (cross-attention block) Trainium2 Bass kernel.

Problem: b=8, c=64, h=w=48 (n=2304), CR=8.
  qk_i = Wqk_i @ x_i + bqk_i  (q = first 8 rows, k = last 8)
  attn_i = softmax_j(q_i^T k_i)            [n, n]
  o1 = (Wv1@x1 + bv1) @ attn2 * gamma + x1
  o2 = (Wv2@x2 + bv2) @ attn1 * beta  + x2

Sharding: data-parallel over batch, 1 batch element per NeuronCore (8 cores).

Per-core dataflow (channel-on-partition):
  - x packed [128, 2304] fp32r: x1 @ partitions 0:64, x2 @ 64:128.
  - q/k [128, 2*2304] fp32r at partitions 0:8 (attn1 cols 0:N, attn2 cols
    N:2N).  Logit matmuls (K=8) run in fp32r (full PE rate for N>=256, vs
    4x slower plain fp32); fp32r requires dst psum partition base 0.
  - A-row (128 queries x 2304 keys) computed in 512-col PSUM chunks through
    3 rotating single-bank tiles; ScalarE exp's each chunk PSUM->SBUF(bf16)
    with accum_out giving softmax row-sums for free.  No max subtraction
    (logit range is far inside fp32 exp range).
  - 1/s (and gamma/beta) folded into the tiny [128, 64] V^T tiles instead of
    the n x n matrix.  E and V^T in bf16 -> o-matmuls run at full rate and
    may use column tiling (o2 at psum partitions 64:128); the attention term
    is scaled by gamma/beta = 0.1, so bf16 rounding there is ~5e-4 relative
    to the residual-dominated output.
  - o1/o2 column-packed into one PSUM accumulator [128, 2304] (5 banks),
    accumulated over all 18 i-tiles via start/stop; final residual add on DVE.
"""

import numpy as np

C = 64
CR = 8
H = W = 48
N = H * W            # 2304
B = 8
P = 128
IT = N // P          # 18 i-tiles
CHUNKS = [(2048, 256), (0, 512), (512, 512), (1024, 512), (1536, 512)]
NCHUNK = len(CHUNKS)

_CACHE = {}


def _build():
    import concourse.bacc as bacc
    import concourse.tile as tile
    from concourse import mybir

    F32 = mybir.dt.float32
    F32R = mybir.dt.float32r
    BF16 = mybir.dt.bfloat16
    AF = mybir.ActivationFunctionType
    ALU = mybir.AluOpType
    AX = mybir.AxisListType

    nc = bacc.Bacc("TRN2", target_bir_lowering=False, debug=False, num_devices=8)

    x1_d = nc.dram_tensor("x1", [C, N], F32R, kind="ExternalInput")
    x2_d = nc.dram_tensor("x2", [C, N], F32R, kind="ExternalInput")
    # consts columns: 0:8 wqT, 8:16 wkT, 16:80 wvT, 80 q1bias, 81 k1bias,
    # 82 q2bias, 83 k2bias (rows 0:8), 84:148 bv1 bcast, 148:212 bv2 bcast,
    # 212 gamma, 213 beta
    cst_d = nc.dram_tensor("consts", [P, 214], F32R, kind="ExternalInput")
    out_d = nc.dram_tensor("out", [P, N], F32, kind="ExternalOutput")

    with tile.TileContext(nc) as tc:
        with (
            tc.tile_pool(name="big", bufs=1) as big,
            tc.tile_pool(name="epool", bufs=3) as epool,
            tc.tile_pool(name="small", bufs=4) as small,
            tc.tile_pool(name="psA", bufs=3, space="PSUM") as psA,
            tc.tile_pool(name="psO", bufs=1, space="PSUM") as psO,
        ):
            # ---- early ACT table warm (loads exp tables during DMA wait)
            warm = big.tile([P, 1], F32, name="warm", tag="warm")
            warm2 = big.tile([P, 1], F32, name="warm2", tag="warm2")
            nc.vector.memset(warm, 0.0)
            nc.scalar.activation(out=warm2, in_=warm, func=AF.Exp)

            # ---- constant + input DMAs
            cst = big.tile([P, 214], F32R, name="cst", tag="cst")
            nc.sync.dma_start(out=cst, in_=cst_d.ap())
            x_sb = big.tile([P, N], F32R, name="x_sb", tag="x_sb")
            nc.sync.dma_start(out=x_sb[0:C, :], in_=x1_d.ap())
            nc.sync.dma_start(out=x_sb[C:P, :], in_=x2_d.ap())

            wq = cst[:, 0:8]
            wk = cst[:, 8:16]
            wv = cst[:, 16:80]
            qkbias = [[cst[:, 80:81].bitcast(F32), cst[:, 81:82].bitcast(F32)],
                      [cst[:, 82:83].bitcast(F32), cst[:, 83:84].bitcast(F32)]]
            bvb = [cst[:, 84:148].bitcast(F32), cst[:, 148:212].bitcast(F32)]
            gamma = cst[:, 212:213].bitcast(F32)
            beta = cst[:, 213:214].bitcast(F32)

            # q/k for both attns at partitions 0:8; attn an at cols an*N
            q_sb = big.tile([P, 2 * N], F32R, name="q_sb", tag="q_sb")
            k_sb = big.tile([P, 2 * N], F32R, name="k_sb", tag="k_sb")
            vt1b = big.tile([P, IT * C], F32, name="vt1b", tag="vt1b")
            vt2b = big.tile([P, IT * C], F32, name="vt2b", tag="vt2b")
            out_sb = big.tile([P, N], F32, name="out_sb", tag="out_sb")

            # ---- projection phase per column chunk: q/k then V^T tiles
            for ci, (off, w) in enumerate(CHUNKS):
                for an in (0, 1):  # attn index
                    xs = x_sb[0:C, off:off + w] if an == 0 else x_sb[C:P, off:off + w]
                    wqs = wq[0:C, :] if an == 0 else wq[C:P, :]
                    wks = wk[0:C, :] if an == 0 else wk[C:P, :]
                    pq = psA.tile([P, 512], F32, name=f"pq{ci}_{an}", tag="ps")
                    nc.tensor.matmul(pq[0:8, :w], wqs, xs)
                    nc.vector.tensor_scalar(
                        out=q_sb[0:8, an * N + off:an * N + off + w],
                        in0=pq[0:8, :w],
                        scalar1=qkbias[an][0][0:8, :], scalar2=None, op0=ALU.add)
                    pk = psA.tile([P, 512], F32, name=f"pk{ci}_{an}", tag="ps")
                    nc.tensor.matmul(pk[0:8, :w], wks, xs)
                    nc.vector.tensor_scalar(
                        out=k_sb[0:8, an * N + off:an * N + off + w],
                        in0=pk[0:8, :w],
                        scalar1=qkbias[an][1][0:8, :], scalar2=None, op0=ALU.add)
                # V^T tiles living in this column chunk
                for t in range(4 * ci, min(4 * ci + 4, IT)):
                    sub = (t % 4) * P + off - (t // 4) * 512
                    sub = (t * P) - off
                    pv1 = psA.tile([P, 512], F32, name=f"pv1_{t}", tag="ps")
                    nc.tensor.matmul(
                        pv1[:, 0:C], x_sb[0:C, t * P:(t + 1) * P], wv[0:C, :])
                    nc.vector.tensor_tensor(
                        out=vt1b[:, t * C:(t + 1) * C], in0=pv1[:, 0:C], in1=bvb[0],
                        op=ALU.add)
                    pv2 = psA.tile([P, 512], F32, name=f"pv2_{t}", tag="ps")
                    nc.tensor.matmul(
                        pv2[:, 0:C], x_sb[C:P, t * P:(t + 1) * P], wv[C:P, :])
                    nc.vector.tensor_tensor(
                        out=vt2b[:, t * C:(t + 1) * C], in0=pv2[:, 0:C], in1=bvb[1],
                        op=ALU.add)

            # ---- main loop over i-tiles
            psum_o = psO.tile([P, N], F32, name="psum_o", tag="pso")

            def emit_omms(t, e1t, e2t, vts):
                st, sp = (t == 0), (t == IT - 1)
                for (off, w) in CHUNKS:
                    # o1 (partitions 0:64) <- vt1s^T @ E2 ; o2 <- vt2s^T @ E1
                    nc.tensor.matmul(
                        psum_o[0:C, off:off + w], vts[:, 0:C],
                        e2t[:, off:off + w], start=st, stop=sp)
                    nc.tensor.matmul(
                        psum_o[C:P, off:off + w], vts[:, C:P],
                        e1t[:, off:off + w], start=st, stop=sp)

            # o-mm emission counts after each of the 10 (an, chunk) A-mm
            # positions: keep PE just ahead of ACT, never a block of o-mms.
            O_COUNTS = [0, 0, 1, 1, 1, 1, 1, 1, 2, 2]

            def omm_list(t, e1t, e2t, vts):
                st, sp = (t == 0), (t == IT - 1)
                mms = []
                for (off, w) in CHUNKS:
                    mms.append((psum_o[0:C, off:off + w], vts[:, 0:C],
                                e2t[:, off:off + w], st, sp))
                    mms.append((psum_o[C:P, off:off + w], vts[:, C:P],
                                e1t[:, off:off + w], st, sp))
                return mms

            prev = None
            for t in range(IT):
                pending = omm_list(*prev) if prev is not None else []
                pi = 0
                ets = []
                sps = []
                for an in (0, 1):
                    qs = slice(an * N + t * P, an * N + (t + 1) * P)
                    et = epool.tile([P, N], BF16, name=f"e{an}_{t}", tag=f"e{an}")
                    sp = small.tile([P, 8], F32, name=f"sp{an}_{t}", tag=f"sp{an}")
                    for ci, (off, w) in enumerate(CHUNKS):
                        pa = psA.tile([P, 512], F32, name=f"pa{an}_{t}_{ci}", tag="ps")
                        nc.tensor.matmul(
                            pa[:, :w], q_sb[0:8, qs],
                            k_sb[0:8, an * N + off:an * N + off + w])
                        nc.scalar.activation(
                            out=et[:, off:off + w], in_=pa[:, :w], func=AF.Exp,
                            accum_out=sp[:, ci:ci + 1])
                        for _ in range(O_COUNTS[an * NCHUNK + ci]):
                            if pending:
                                o, l, rr_, st_, sp_ = pending.pop(0)
                                nc.tensor.matmul(o, l, rr_, start=st_, stop=sp_)
                    ets.append(et)
                    sps.append(sp)
                    # row stats as soon as this row's exps are emitted
                    s = small.tile([P, 1], F32, name=f"s{an}_{t}", tag=f"s{an}")
                    nc.vector.tensor_reduce(
                        s, sp[:, 0:NCHUNK], axis=AX.X, op=ALU.add)
                    rr = small.tile([P, 1], F32, name=f"r{an}_{t}", tag=f"r{an}")
                    nc.vector.reciprocal(rr, s)
                    sps.append(rr)
                for o, l, rr_, st_, sp_ in pending:
                    nc.tensor.matmul(o, l, rr_, start=st_, stop=sp_)
                r1_, r2_ = sps[1], sps[3]
                # vts cols 0:64 = vt1b*(1/s2)*gamma ; 64:128 = vt2b*(1/s1)*beta
                vts = small.tile([P, P], BF16, name=f"vts_{t}", tag="vts")
                nc.vector.tensor_scalar(
                    out=vts[:, 0:C], in0=vt1b[:, t * C:(t + 1) * C],
                    scalar1=r2_, scalar2=gamma, op0=ALU.mult, op1=ALU.mult)
                nc.vector.tensor_scalar(
                    out=vts[:, C:P], in0=vt2b[:, t * C:(t + 1) * C],
                    scalar1=r1_, scalar2=beta, op0=ALU.mult, op1=ALU.mult)
                prev = (t, ets[0], ets[1], vts)
            for o, l, rr_, st_, sp_ in omm_list(*prev):
                nc.tensor.matmul(o, l, rr_, start=st_, stop=sp_)

            # ---- final: out = psum_o + x  (residual), chunked store
            for ci, (off, w) in enumerate(CHUNKS):
                nc.vector.tensor_tensor(
                    out=out_sb[:, off:off + w], in0=psum_o[:, off:off + w],
                    in1=x_sb[:, off:off + w].bitcast(F32), op=ALU.add)
                nc.sync.dma_start(
                    out=out_d.ap()[:, off:off + w], in_=out_sb[:, off:off + w])

    nc.compile()
    return nc


def _get_nc():
    if "nc" not in _CACHE:
        _CACHE["nc"] = _build()
    return _CACHE["nc"]


def _make_in_maps(x1, x2, Wqk1, bqk1, Wqk2, bqk2, Wv1, bv1, Wv2, bv2, gamma, beta):
    f = np.float32
    consts = np.zeros((P, 214), dtype=f)
    consts[0:C, 0:8] = np.asarray(Wqk1, f)[0:CR, :].T
    consts[C:P, 0:8] = np.asarray(Wqk2, f)[0:CR, :].T
    consts[0:C, 8:16] = np.asarray(Wqk1, f)[CR:2 * CR, :].T
    consts[C:P, 8:16] = np.asarray(Wqk2, f)[CR:2 * CR, :].T
    consts[0:C, 16:80] = np.asarray(Wv1, f).T
    consts[C:P, 16:80] = np.asarray(Wv2, f).T
    consts[0:CR, 80] = np.asarray(bqk1, f)[0:CR]
    consts[0:CR, 81] = np.asarray(bqk1, f)[CR:2 * CR]
    consts[0:CR, 82] = np.asarray(bqk2, f)[0:CR]
    consts[0:CR, 83] = np.asarray(bqk2, f)[CR:2 * CR]
    consts[:, 84:148] = np.asarray(bv1, f)[None, :]
    consts[:, 148:212] = np.asarray(bv2, f)[None, :]
    consts[:, 212] = np.float32(np.asarray(gamma, f).reshape(-1)[0])
    consts[:, 213] = np.float32(np.asarray(beta, f).reshape(-1)[0])

    x1 = np.ascontiguousarray(np.asarray(x1, f).reshape(B, C, N))
    x2 = np.ascontiguousarray(np.asarray(x2, f).reshape(B, C, N))
    return [
        {"x1": np.ascontiguousarray(x1[i]), "x2": np.ascontiguousarray(x2[i]),
         "consts": consts}
        for i in range(B)
    ]


def _run(in_maps, **kwargs):
    from concourse.bass_utils import run_bass_kernel_spmd
    nc = _get_nc()
    return run_bass_kernel_spmd(nc, in_maps, core_ids=list(range(B)), **kwargs)


def kernel(x1, x2, Wqk1, bqk1, Wqk2, bqk2, Wv1, bv1, Wv2, bv2, gamma, beta):
    in_maps = _make_in_maps(x1, x2, Wqk1, bqk1, Wqk2, bqk2, Wv1, bv1, Wv2, bv2,
                            gamma, beta)
    res = _run(in_maps)
    o1 = np.empty((B, C, H, W), dtype=np.float32)
    o2 = np.empty((B, C, H, W), dtype=np.float32)
    for i in range(B):
        full = res.results[i]["out"]
        o1[i] = full[0:C, :].reshape(C, H, W)
        o2[i] = full[C:P, :].reshape(C, H, W)
    return o1, o2


# revision 17
# speedup vs baseline: 1.1222x; 1.1214x over previous
"""CABlock (cross-attention block) Trainium2 Bass kernel.

Problem: b=8, c=64, h=w=48 (n=2304), CR=8.
  qk_i = Wqk_i @ x_i + bqk_i  (q = first 8 rows, k = last 8)
  attn_i = softmax_j(q_i^T k_i)            [n, n]
  o1 = (Wv1@x1 + bv1) @ attn2 * gamma + x1
  o2 = (Wv2@x2 + bv2) @ attn1 * beta  + x2

Sharding: data-parallel over batch, 1 batch element per NeuronCore (8 cores).

Per-core dataflow (channel-on-partition):
  - x packed [128, 2304] fp32r: x1 @ partitions 0:64, x2 @ 64:128.
  - q/k [128, 2*2304] fp32r at partitions 0:8 (attn1 cols 0:N, attn2 cols
    N:2N).  Logit matmuls (K=8) run in fp32r (full PE rate for N>=256, vs
    4x slower plain fp32); fp32r requires dst psum partition base 0.
  - A-row (128 queries x 2304 keys) computed in 512-col PSUM chunks through
    3 rotating single-bank tiles; ScalarE exp's each chunk PSUM->SBUF(bf16)
    with accum_out giving softmax row-sums for free.  No max subtraction
    (logit range is far inside fp32 exp range).
  - 1/s (and gamma/beta) folded into the tiny [128, 64] V^T tiles instead of
    the n x n matrix.  E and V^T in bf16 -> o-matmuls run at full rate and
    may use column tiling (o2 at psum partitions 64:128); the attention term
    is scaled by gamma/beta = 0.1, so bf16 rounding there is ~5e-4 relative
    to the residual-dominated output.
  - o1/o2 column-packed into one PSUM accumulator [128, 2304] (5 banks),
    accumulated over all 18 i-tiles via start/stop; final residual add on DVE.
"""

import numpy as np

C = 64
CR = 8
H = W = 48
N = H * W            # 2304
B = 8
P = 128
IT = N // P          # 18 i-tiles
CHUNKS = [(2048, 256), (0, 512), (512, 512), (1024, 512), (1536, 512)]
NCHUNK = len(CHUNKS)

_CACHE = {}


def _build():
    import concourse.bacc as bacc
    import concourse.tile as tile
    from concourse import mybir

    F32 = mybir.dt.float32
    F32R = mybir.dt.float32r
    BF16 = mybir.dt.bfloat16
    AF = mybir.ActivationFunctionType
    ALU = mybir.AluOpType
    AX = mybir.AxisListType

    nc = bacc.Bacc("TRN2", target_bir_lowering=False, debug=False, num_devices=8)

    x1_d = nc.dram_tensor("x1", [C, N], F32R, kind="ExternalInput")
    x2_d = nc.dram_tensor("x2", [C, N], F32R, kind="ExternalInput")
    # consts columns: 0:8 wqT, 8:16 wkT, 16:80 wvT, 80 q1bias, 81 k1bias,
    # 82 q2bias, 83 k2bias (rows 0:8), 84:148 bv1 bcast, 148:212 bv2 bcast,
    # 212 gamma, 213 beta
    cst_d = nc.dram_tensor("consts", [P, 214], F32R, kind="ExternalInput")
    out_d = nc.dram_tensor("out", [P, N], F32, kind="ExternalOutput")

    with tile.TileContext(nc) as tc:
        with (
            tc.tile_pool(name="big", bufs=1) as big,
            tc.tile_pool(name="epool", bufs=3) as epool,
            tc.tile_pool(name="small", bufs=4) as small,
            tc.tile_pool(name="psA", bufs=3, space="PSUM") as psA,
            tc.tile_pool(name="psO", bufs=1, space="PSUM") as psO,
        ):
            # ---- early ACT table warm (loads exp tables during DMA wait)
            warm = big.tile([P, 1], F32, name="warm", tag="warm")
            warm2 = big.tile([P, 1], F32, name="warm2", tag="warm2")
            nc.vector.memset(warm, 0.0)
            nc.scalar.activation(out=warm2, in_=warm, func=AF.Exp)

            # ---- constant + input DMAs
            cst = big.tile([P, 214], F32R, name="cst", tag="cst")
            nc.sync.dma_start(out=cst, in_=cst_d.ap())
            x_sb = big.tile([P, N], F32R, name="x_sb", tag="x_sb")
            nc.sync.dma_start(out=x_sb[0:C, :], in_=x1_d.ap())
            nc.sync.dma_start(out=x_sb[C:P, :], in_=x2_d.ap())

            wq = cst[:, 0:8]
            wk = cst[:, 8:16]
            wv = cst[:, 16:80]
            qkbias = [[cst[:, 80:81].bitcast(F32), cst[:, 81:82].bitcast(F32)],
                      [cst[:, 82:83].bitcast(F32), cst[:, 83:84].bitcast(F32)]]
            bvb = [cst[:, 84:148].bitcast(F32), cst[:, 148:212].bitcast(F32)]
            gamma = cst[:, 212:213].bitcast(F32)
            beta = cst[:, 213:214].bitcast(F32)

            # ---- PE HAM warm-up: ~3.4us of dummy matmuls during DMA wait
            wz = big.tile([P, 512], BF16, name="wz", tag="wz")
            nc.vector.memset(wz, 0.0)
            wps = psA.tile([P, 512], F32, name="wps", tag="ps")
            for _wi in range(16):
                nc.tensor.matmul(wps[:, 0:512], wz[:, 0:128], wz[:, 0:512])

            # q/k for both attns at partitions 0:8; attn an at cols an*N
            q_sb = big.tile([P, 2 * N], F32R, name="q_sb", tag="q_sb")
            k_sb = big.tile([P, 2 * N], F32R, name="k_sb", tag="k_sb")
            vt1b = big.tile([P, IT * C], F32, name="vt1b", tag="vt1b")
            vt2b = big.tile([P, IT * C], F32, name="vt2b", tag="vt2b")
            out_sb = big.tile([P, N], F32, name="out_sb", tag="out_sb")

            # ---- projection phase per column chunk: q/k then V^T tiles
            for ci, (off, w) in enumerate(CHUNKS):
                for an in (0, 1):  # attn index
                    xs = x_sb[0:C, off:off + w] if an == 0 else x_sb[C:P, off:off + w]
                    wqs = wq[0:C, :] if an == 0 else wq[C:P, :]
                    wks = wk[0:C, :] if an == 0 else wk[C:P, :]
                    pq = psA.tile([P, 512], F32, name=f"pq{ci}_{an}", tag="ps")
                    nc.tensor.matmul(pq[0:8, :w], wqs, xs)
                    nc.vector.tensor_scalar(
                        out=q_sb[0:8, an * N + off:an * N + off + w],
                        in0=pq[0:8, :w],
                        scalar1=qkbias[an][0][0:8, :], scalar2=None, op0=ALU.add)
                    pk = psA.tile([P, 512], F32, name=f"pk{ci}_{an}", tag="ps")
                    nc.tensor.matmul(pk[0:8, :w], wks, xs)
                    nc.vector.tensor_scalar(
                        out=k_sb[0:8, an * N + off:an * N + off + w],
                        in0=pk[0:8, :w],
                        scalar1=qkbias[an][1][0:8, :], scalar2=None, op0=ALU.add)
                # V^T tiles living in this column chunk
                for t in range(4 * ci, min(4 * ci + 4, IT)):
                    sub = (t % 4) * P + off - (t // 4) * 512
                    sub = (t * P) - off
                    pv1 = psA.tile([P, 512], F32, name=f"pv1_{t}", tag="ps")
                    nc.tensor.matmul(
                        pv1[:, 0:C], x_sb[0:C, t * P:(t + 1) * P], wv[0:C, :])
                    nc.vector.tensor_tensor(
                        out=vt1b[:, t * C:(t + 1) * C], in0=pv1[:, 0:C], in1=bvb[0],
                        op=ALU.add)
                    pv2 = psA.tile([P, 512], F32, name=f"pv2_{t}", tag="ps")
                    nc.tensor.matmul(
                        pv2[:, 0:C], x_sb[C:P, t * P:(t + 1) * P], wv[C:P, :])
                    nc.vector.tensor_tensor(
                        out=vt2b[:, t * C:(t + 1) * C], in0=pv2[:, 0:C], in1=bvb[1],
                        op=ALU.add)

            # ---- main loop over i-tiles
            psum_o = psO.tile([P, N], F32, name="psum_o", tag="pso")

            def emit_omms(t, e1t, e2t, vts):
                st, sp = (t == 0), (t == IT - 1)
                for (off, w) in CHUNKS:
                    # o1 (partitions 0:64) <- vt1s^T @ E2 ; o2 <- vt2s^T @ E1
                    nc.tensor.matmul(
                        psum_o[0:C, off:off + w], vts[:, 0:C],
                        e2t[:, off:off + w], start=st, stop=sp)
                    nc.tensor.matmul(
                        psum_o[C:P, off:off + w], vts[:, C:P],
                        e1t[:, off:off + w], start=st, stop=sp)

            # o-mm emission counts after each of the 10 (an, chunk) A-mm
            # positions: keep PE just ahead of ACT, never a block of o-mms.
            O_COUNTS = [0, 0, 2, 2, 2, 2, 2, 0, 0, 0]

            def omm_list(t, e1t, e2t, vts):
                st, sp = (t == 0), (t == IT - 1)
                mms = []
                for (off, w) in CHUNKS:
                    mms.append((psum_o[0:C, off:off + w], vts[:, 0:C],
                                e2t[:, off:off + w], st, sp))
                    mms.append((psum_o[C:P, off:off + w], vts[:, C:P],
                                e1t[:, off:off + w], st, sp))
                return mms

            prev = None
            for t in range(IT):
                pending = omm_list(*prev) if prev is not None else []
                pi = 0
                ets = []
                sps = []
                for an in (0, 1):
                    qs = slice(an * N + t * P, an * N + (t + 1) * P)
                    et = epool.tile([P, N], BF16, name=f"e{an}_{t}", tag=f"e{an}")
                    sp = small.tile([P, 8], F32, name=f"sp{an}_{t}", tag=f"sp{an}")
                    for ci, (off, w) in enumerate(CHUNKS):
                        pa = psA.tile([P, 512], F32, name=f"pa{an}_{t}_{ci}", tag="ps")
                        nc.tensor.matmul(
                            pa[:, :w], q_sb[0:8, qs],
                            k_sb[0:8, an * N + off:an * N + off + w])
                        nc.scalar.activation(
                            out=et[:, off:off + w], in_=pa[:, :w], func=AF.Exp,
                            accum_out=sp[:, ci:ci + 1])
                        for _ in range(O_COUNTS[an * NCHUNK + ci]):
                            if pending:
                                o, l, rr_, st_, sp_ = pending.pop(0)
                                nc.tensor.matmul(o, l, rr_, start=st_, stop=sp_)
                    ets.append(et)
                    sps.append(sp)
                    # row stats as soon as this row's exps are emitted
                    s = small.tile([P, 1], F32, name=f"s{an}_{t}", tag=f"s{an}")
                    nc.vector.tensor_reduce(
                        s, sp[:, 0:NCHUNK], axis=AX.X, op=ALU.add)
                    rr = small.tile([P, 1], F32, name=f"r{an}_{t}", tag=f"r{an}")
                    nc.vector.reciprocal(rr, s)
                    sps.append(rr)
                for o, l, rr_, st_, sp_ in pending:
                    nc.tensor.matmul(o, l, rr_, start=st_, stop=sp_)
                r1_, r2_ = sps[1], sps[3]
                # vts cols 0:64 = vt1b*(1/s2)*gamma ; 64:128 = vt2b*(1/s1)*beta
                vts = small.tile([P, P], BF16, name=f"vts_{t}", tag="vts")
                nc.vector.tensor_scalar(
                    out=vts[:, 0:C], in0=vt1b[:, t * C:(t + 1) * C],
                    scalar1=r2_, scalar2=gamma, op0=ALU.mult, op1=ALU.mult)
                nc.vector.tensor_scalar(
                    out=vts[:, C:P], in0=vt2b[:, t * C:(t + 1) * C],
                    scalar1=r1_, scalar2=beta, op0=ALU.mult, op1=ALU.mult)
                prev = (t, ets[0], ets[1], vts)
            for o, l, rr_, st_, sp_ in omm_list(*prev):
                nc.tensor.matmul(o, l, rr_, start=st_, stop=sp_)

            # ---- final: out = psum_o + x  (residual), chunked store
            for ci, (off, w) in enumerate(CHUNKS):
                nc.vector.tensor_tensor(
                    out=out_sb[:, off:off + w], in0=psum_o[:, off:off + w],
                    in1=x_sb[:, off:off + w].bitcast(F32), op=ALU.add)
                nc.sync.dma_start(
                    out=out_d.ap()[:, off:off + w], in_=out_sb[:, off:off + w])

    nc.compile()
    return nc


def _get_nc():
    if "nc" not in _CACHE:
        _CACHE["nc"] = _build()
    return _CACHE["nc"]


def _make_in_maps(x1, x2, Wqk1, bqk1, Wqk2, bqk2, Wv1, bv1, Wv2, bv2, gamma, beta):
    f = np.float32
    consts = np.zeros((P, 214), dtype=f)
    consts[0:C, 0:8] = np.asarray(Wqk1, f)[0:CR, :].T
    consts[C:P, 0:8] = np.asarray(Wqk2, f)[0:CR, :].T
    consts[0:C, 8:16] = np.asarray(Wqk1, f)[CR:2 * CR, :].T
    consts[C:P, 8:16] = np.asarray(Wqk2, f)[CR:2 * CR, :].T
    consts[0:C, 16:80] = np.asarray(Wv1, f).T
    consts[C:P, 16:80] = np.asarray(Wv2, f).T
    consts[0:CR, 80] = np.asarray(bqk1, f)[0:CR]
    consts[0:CR, 81] = np.asarray(bqk1, f)[CR:2 * CR]
    consts[0:CR, 82] = np.asarray(bqk2, f)[0:CR]
    consts[0:CR, 83] = np.asarray(bqk2, f)[CR:2 * CR]
    consts[:, 84:148] = np.asarray(bv1, f)[None, :]
    consts[:, 148:212] = np.asarray(bv2, f)[None, :]
    consts[:, 212] = np.float32(np.asarray(gamma, f).reshape(-1)[0])
    consts[:, 213] = np.float32(np.asarray(beta, f).reshape(-1)[0])

    x1 = np.ascontiguousarray(np.asarray(x1, f).reshape(B, C, N))
    x2 = np.ascontiguousarray(np.asarray(x2, f).reshape(B, C, N))
    return [
        {"x1": np.ascontiguousarray(x1[i]), "x2": np.ascontiguousarray(x2[i]),
         "consts": consts}
        for i in range(B)
    ]


def _run(in_maps, **kwargs):
    from concourse.bass_utils import run_bass_kernel_spmd
    nc = _get_nc()
    return run_bass_kernel_spmd(nc, in_maps, core_ids=list(range(B)), **kwargs)


def kernel(x1, x2, Wqk1, bqk1, Wqk2, bqk2, Wv1, bv1, Wv2, bv2, gamma, beta):
    in_maps = _make_in_maps(x1, x2, Wqk1, bqk1, Wqk2, bqk2, Wv1, bv1, Wv2, bv2,
                            gamma, beta)
    res = _run(in_maps)
    o1 = np.empty((B, C, H, W), dtype=np.float32)
    o2 = np.empty((B, C, H, W), dtype=np.float32)
    for i in range(B):
        full = res.results[i]["out"]
        o1[i] = full[0:C, :].reshape(C, H, W)
        o2[i] = full[C:P, :].reshape(C, H, W)
    return o1, o2
